# revision 11
# baseline (speedup 1.0000x reference)
"""Mixture-of-Experts (top-2 of 8) Trainium2 Bass kernel.

Strategy: data-parallel over tokens — each of the 8 NeuronCores owns a
contiguous shard of 4096 tokens and computes its shard's full output.

Per core, on device:
  Phase R (routing): per 128-token tile, compute router logits on PE
    (PE-transpose + matmul), derive top-2 combine weights (softmax
    renormalized over the top-2 — the full-softmax denominator cancels),
    compute each token's slot in its experts' dispatch buffers with a
    matmul prefix-sum (strict-upper-triangular ones matrix + running
    base), then indirect-DMA-scatter the fp16 token row into per-expert
    compact buffers `xd` and (weight, token-id) rows into `meta`.
  Phase E (experts): per expert, DMA-transpose-load dispatched tokens as
    xT [d, slot], hT = gelu(W1.T @ xT + b1) with W1 streamed in j-blocks,
    out = hT.T @ W2 + b2 with W2 resident in halves, scale rows by the
    combine weight, indirect-DMA-scatter-accumulate into `out`.

Output row 0 is a trash row: pad slots carry id=0 / weight=0, so their
garbage lands there. Token t lives at out row 1+t. Biases are applied as
K=1 rank-1 matmuls. Weights are pre-packed and pre-cast to fp16 blocked
layouts on the host so every device weight DMA is contiguous per
partition.
"""

import os
import sys
from contextlib import ExitStack

import numpy as np

for _p in ("/opt/trn_rl_repo", "/root/.axon_site/_ro/trn_rl_repo"):
    if os.path.isdir(_p) and _p not in sys.path:
        sys.path.append(_p)

import concourse.bass as bass
import concourse.bacc as bacc
import concourse.tile as tile
from concourse import mybir
from concourse.masks import make_identity, make_upper_triangular

F32 = mybir.dt.float32
F16 = mybir.dt.float16
I32 = mybir.dt.int32
P = 128


def build_moe(T, D, H, E, C, act="Gelu", nd=256, mc=512):
    """Per-core MoE program. T tokens, D model, H hidden, E experts,
    C capacity/expert (mult of 128). nd = W2 d-chunk, mc = mm1 token chunk."""
    assert T % P == 0 and D % P == 0 and H % P == 0 and C % P == 0
    DK, HJ, SC, NT, NH = D // P, H // P, C // P, T // P, D // nd
    act_fn = getattr(mybir.ActivationFunctionType, act)

    nc = bacc.Bacc("TRN2", target_bir_lowering=False, debug=False)

    x = nc.declare_dram_parameter("x", [T, D], F32, isOutput=False)
    wrp = nc.declare_dram_parameter("wrp", [P, DK, E], F32, isOutput=False)
    br = nc.declare_dram_parameter("br", [E], F32, isOutput=False)
    w1p = nc.declare_dram_parameter("w1p", [E, HJ, P, DK, P], F16, isOutput=False)
    b1p = nc.declare_dram_parameter("b1p", [E, P, HJ], F32, isOutput=False)
    w2p = nc.declare_dram_parameter("w2p", [E, NH, P, HJ, nd], F16, isOutput=False)
    b2 = nc.declare_dram_parameter("b2", [E, D], F32, isOutput=False)
    out = nc.declare_dram_parameter("out", [1 + T + P, D], F32, isOutput=True)

    xd = nc.dram_tensor("xd", [E * C, D], F16)        # dispatched tokens
    meta = nc.dram_tensor("meta", [E * C, 2], F32)    # (combine w, 1 + token id)

    with ExitStack() as ctx:
        tc = ctx.enter_context(tile.TileContext(nc))
        cst = ctx.enter_context(tc.tile_pool(name="cst", bufs=1))
        pR2 = ctx.enter_context(tc.tile_pool(name="pR2", bufs=2))
        psA = ctx.enter_context(tc.tile_pool(name="psA", bufs=3, space="PSUM"))
        psB = ctx.enter_context(tc.tile_pool(name="psB", bufs=3, space="PSUM"))
        xt1 = ctx.enter_context(tc.tile_pool(name="xt1", bufs=1))
        ht1 = ctx.enter_context(tc.tile_pool(name="ht1", bufs=1))
        w1pl = ctx.enter_context(tc.tile_pool(name="w1pl", bufs=3))
        w2pl = ctx.enter_context(tc.tile_pool(name="w2pl", bufs=2))
        otp = ctx.enter_context(tc.tile_pool(name="otp", bufs=3))
        sm2 = ctx.enter_context(tc.tile_pool(name="sm2", bufs=2))

        # ---- constants ----
        su = cst.tile([P, P], F32)  # (made before ident: PE's first pool-wait covers both)  # su[m, n] = 1 iff m < n
        make_upper_triangular(nc, su[:], val=1.0, diag=False)
        ident = cst.tile([P, P], F32)
        make_identity(nc, ident[:])
        ones_r = cst.tile([1, max(P, nd, mc, E)], F32)
        nc.vector.memset(ones_r[:], 1.0)
        ones_c = cst.tile([P, 1], F32)
        nc.vector.memset(ones_c[:], 1.0)
        ecol = cst.tile([P, E], F32)  # ecol[p, e] = e * C
        nc.gpsimd.iota(ecol[:], pattern=[[1, E]], base=0, channel_multiplier=0,
                       allow_small_or_imprecise_dtypes=True)
        nc.vector.tensor_scalar_mul(out=ecol[:], in0=ecol[:], scalar1=float(C))
        wr_sb = cst.tile([P, DK, E], F32)
        nc.sync.dma_start(out=wr_sb[:], in_=wrp[:])
        br_sb = cst.tile([1, E], F32)
        nc.sync.dma_start(out=br_sb[:1, :], in_=br[None, :])
        zt = cst.tile([P, D], F32)
        nc.vector.memset(zt[:], 0.0)

        # ---- zero-init out and meta ----
        n_zero = (1 + T + P + P - 1) // P
        for i in range(n_zero):
            r0 = i * P
            r1_ = min(r0 + P, 1 + T + P)
            nc.sync.dma_start(out=out[r0:r1_, :], in_=zt[: r1_ - r0, :])
        npp = E * C // P  # meta rows per partition
        nc.sync.dma_start(out=meta[:].rearrange("(p n) c -> p (n c)", p=P),
                          in_=zt[:, : npp * 2])
        # token ids for all tiles: tok_all[p, it] = 1 + it*128 + p
        tok_all = cst.tile([P, 512], I32)
        nc.gpsimd.iota(tok_all[:], pattern=[[P, 512]], base=1, channel_multiplier=1)
        zt16 = cst.tile([P, D], F16)
        nc.vector.memset(zt16[:], 0.0)
        for i in range(E * C // P):
            nc.sync.dma_start(out=xd[i * P:(i + 1) * P, :], in_=zt16[:])

        # ---- Phase R: routing + dispatch ----
        bc_reg = nc.gpsimd.to_reg(E * C - 1)
        base_prev = None
        for it in range(NT):
            x_t = pR2.tile([P, D], F32, tag="x_t")
            nc.sync.dma_start(out=x_t[:], in_=x[it * P:(it + 1) * P, :])
            x16 = pR2.tile([P, D], F16, tag="x16")
            nc.vector.tensor_copy(out=x16[:], in_=x_t[:])

            xTr = pR2.tile([P, DK, P], F32, tag="xTr")
            for dk in range(DK):
                tr_ps = psA.tile([P, P], F32, tag="mmout", space="PSUM")
                nc.tensor.transpose(out=tr_ps[:], in_=x_t[:, dk * P:(dk + 1) * P],
                                    identity=ident[:])
                nc.vector.tensor_copy(out=xTr[:, dk, :], in_=tr_ps[:])

            lg_ps = psB.tile([P, E], F32, tag="small", space="PSUM")
            for dk in range(DK):
                nc.tensor.matmul(lg_ps[:], lhsT=xTr[:, dk, :], rhs=wr_sb[:, dk, :],
                                 start=(dk == 0), stop=False)
            nc.tensor.matmul(lg_ps[:], lhsT=ones_r[:1, :P], rhs=br_sb[:1, :],
                             start=False, stop=True)
            lg = pR2.tile([P, E], F32, tag="lg")
            nc.vector.tensor_copy(out=lg[:], in_=lg_ps[:])

            # top-2 combine weights
            m1 = pR2.tile([P, 1], F32, tag="m1")
            nc.vector.reduce_max(out=m1[:], in_=lg[:], axis=mybir.AxisListType.X)
            is1 = pR2.tile([P, E], F32, tag="is1")
            nc.vector.tensor_scalar(out=is1[:], in0=lg[:], scalar1=m1[:, :1],
                                    scalar2=None, op0=mybir.AluOpType.is_equal)
            lm = pR2.tile([P, E], F32, tag="lm")
            nc.vector.tensor_scalar(out=lm[:], in0=is1[:], scalar1=-1e30, scalar2=None,
                                    op0=mybir.AluOpType.mult)
            nc.vector.tensor_add(out=lm[:], in0=lm[:], in1=lg[:])
            m2 = pR2.tile([P, 1], F32, tag="m2")
            nc.vector.reduce_max(out=m2[:], in_=lm[:], axis=mybir.AxisListType.X)
            m1n = pR2.tile([P, 1], F32, tag="m1n")
            nc.vector.tensor_scalar_mul(out=m1n[:], in0=m1[:], scalar1=-1.0)
            ex = pR2.tile([P, E], F32, tag="ex")
            nc.scalar.activation(out=ex[:], in_=lg[:],
                                 func=mybir.ActivationFunctionType.Exp,
                                 bias=m1n[:, :1], scale=1.0)
            sel = pR2.tile([P, E], F32, tag="sel")
            nc.vector.tensor_scalar(out=sel[:], in0=lg[:], scalar1=m2[:, :1],
                                    scalar2=None, op0=mybir.AluOpType.is_ge)
            w = pR2.tile([P, E], F32, tag="w")
            nc.vector.tensor_mul(out=w[:], in0=ex[:], in1=sel[:])
            den = pR2.tile([P, 1], F32, tag="den")
            nc.vector.reduce_sum(out=den[:], in_=w[:], axis=mybir.AxisListType.X)
            rcp = pR2.tile([P, 1], F32, tag="rcp")
            nc.vector.reciprocal(out=rcp[:], in_=den[:])
            comb = pR2.tile([P, E], F32, tag="comb")
            nc.vector.tensor_scalar_mul(out=comb[:], in0=w[:], scalar1=rcp[:, :1])

            # pos[n, e] = base[e] + sum_{m<n in tile} sel[m, e]
            pos_ps = psB.tile([P, E], F32, tag="small", space="PSUM")
            nc.tensor.matmul(pos_ps[:], lhsT=su[:], rhs=sel[:], start=True,
                             stop=(base_prev is None))
            if base_prev is not None:
                nc.tensor.matmul(pos_ps[:], lhsT=ones_r[:1, :P], rhs=base_prev[:1, :],
                                 start=False, stop=True)
            pos = pR2.tile([P, E], F32, tag="pos")
            nc.vector.tensor_copy(out=pos[:], in_=pos_ps[:])

            # base_new = base_prev + colsum(sel)
            cs_ps = psB.tile([1, E], F32, tag="small", space="PSUM")
            nc.tensor.matmul(cs_ps[:1, :], lhsT=ones_c[:, :1], rhs=sel[:],
                             start=True, stop=True)
            base_new = pR2.tile([1, E], F32, tag="base")
            if base_prev is not None:
                nc.vector.tensor_add(out=base_new[:1, :], in0=base_prev[:1, :],
                                     in1=cs_ps[:1, :])
            else:
                nc.vector.tensor_copy(out=base_new[:1, :], in_=cs_ps[:1, :])
            base_prev = base_new

            # offsets (with capacity-overflow guard -> OOB skip)
            offm = pR2.tile([P, E], F32, tag="offm")
            ovf = pR2.tile([P, E], F32, tag="ovf")
            nc.vector.tensor_scalar(out=ovf[:], in0=pos[:], scalar1=float(C) - 0.5,
                                    scalar2=1e9, op0=mybir.AluOpType.is_ge,
                                    op1=mybir.AluOpType.mult)
            nc.vector.tensor_add(out=offm[:], in0=pos[:], in1=ecol[:])
            nc.vector.tensor_add(out=offm[:], in0=offm[:], in1=ovf[:])

            r1 = pR2.tile([P, E], F32, tag="r1")
            nc.vector.tensor_sub(out=r1[:], in0=sel[:], in1=is1[:])

            tokf = pR2.tile([P, 1], F32, tag="tokf")
            nc.vector.tensor_copy(out=tokf[:], in_=tok_all[:, it:it + 1])

            for rank, mask in ((0, is1), (1, r1)):
                tmp = pR2.tile([P, E], F32, tag=f"tmp{rank}")
                offr = pR2.tile([P, 1], F32, tag=f"offr{rank}")
                nc.vector.tensor_mul(out=tmp[:], in0=offm[:], in1=mask[:])
                nc.vector.reduce_sum(out=offr[:], in_=tmp[:], axis=mybir.AxisListType.X)
                offi = pR2.tile([P, 1], I32, tag=f"offi{rank}")
                nc.vector.tensor_copy(out=offi[:], in_=offr[:])
                wr_t = pR2.tile([P, 1], F32, tag=f"wr_t{rank}")
                nc.vector.tensor_mul(out=tmp[:], in0=comb[:], in1=mask[:])
                nc.vector.reduce_sum(out=wr_t[:], in_=tmp[:], axis=mybir.AxisListType.X)
                mt = pR2.tile([P, 2], F32, tag=f"mt{rank}")
                nc.vector.tensor_copy(out=mt[:, 0:1], in_=wr_t[:])
                nc.vector.tensor_copy(out=mt[:, 1:2], in_=tokf[:])
                nc.gpsimd.indirect_dma_start(
                    out=meta[:],
                    out_offset=bass.IndirectOffsetOnAxis(ap=offi[:, :1], axis=0),
                    in_=mt[:], in_offset=None,
                    bounds_check=bc_reg, oob_is_err=False)
                nc.gpsimd.indirect_dma_start(
                    out=xd[:],
                    out_offset=bass.IndirectOffsetOnAxis(ap=offi[:, :1], axis=0),
                    in_=x16[:], in_offset=None,
                    bounds_check=bc_reg, oob_is_err=False)

        # ---- Phase E: per-expert MLP ----
        mchunks = []
        s0 = 0
        while s0 < C:
            mchunks.append((s0, min(mc, C - s0)))
            s0 += mc

        for e in range(E):
            meta_sb = sm2.tile([P, SC, 2], F32, tag="meta_sb")
            nc.sync.dma_start(
                out=meta_sb[:],
                in_=meta[e * C:(e + 1) * C, :].rearrange("(s p) c -> p s c", p=P))
            idx_i = sm2.tile([P, SC], I32, tag="idx_i")
            nc.vector.tensor_copy(out=idx_i[:], in_=meta_sb[:, :, 1])
            b1_sb = sm2.tile([P, HJ], F32, tag="b1_sb")
            nc.sync.dma_start(out=b1_sb[:], in_=b1p[e])
            b2_sb = sm2.tile([1, D], F32, tag="b2_sb")
            nc.sync.dma_start(out=b2_sb[:1, :], in_=b2[e, None, :])

            xT = xt1.tile([P, DK, C], F16, tag="xT")
            for dk in range(DK):
                for s in range(SC):
                    nc.sync.dma_start(
                        out=xT[:, dk, s * P:(s + 1) * P],
                        in_=xd[e * C + s * P:e * C + (s + 1) * P, dk * P:(dk + 1) * P],
                        transpose=True)

            hT = ht1.tile([P, HJ, C], F16, tag="hT")
            for j in range(HJ):
                w1j = w1pl.tile([P, DK, P], F16, tag="w1j")
                nc.sync.dma_start(out=w1j[:], in_=w1p[e, j])
                for (c0, ns) in mchunks:
                    ph = psA.tile([P, mc], F32, tag="mmout", space="PSUM")
                    for dk in range(DK):
                        nc.tensor.matmul(ph[:, :ns], lhsT=w1j[:, dk, :],
                                         rhs=xT[:, dk, c0:c0 + ns],
                                         start=(dk == 0), stop=(dk == DK - 1))
                    nc.scalar.activation(out=hT[:, j, c0:c0 + ns], in_=ph[:, :ns],
                                         func=act_fn, bias=b1_sb[:, j:j + 1], scale=1.0)

            for hi in range(NH):
                h0 = hi * nd
                w2h = w2pl.tile([P, HJ, nd], F16, tag="w2h")
                nc.sync.dma_start(out=w2h[:], in_=w2p[e, hi])
                for s in range(SC):
                    po = psA.tile([P, nd], F32, tag="mmout", space="PSUM")
                    for j in range(HJ):
                        nc.tensor.matmul(po[:], lhsT=hT[:, j, s * P:(s + 1) * P],
                                         rhs=w2h[:, j, :], start=(j == 0), stop=False)
                    nc.tensor.matmul(po[:], lhsT=ones_r[:1, :P],
                                     rhs=b2_sb[:1, h0:h0 + nd], start=False, stop=True)
                    ot = otp.tile([P, nd], F32, tag="ot")
                    nc.vector.tensor_scalar_mul(out=ot[:], in0=po[:],
                                                scalar1=meta_sb[:, s, 0:1])
                    nc.gpsimd.indirect_dma_start(
                        out=out[:],
                        out_offset=bass.IndirectOffsetOnAxis(ap=idx_i[:, s:s + 1], axis=0),
                        in_=ot[:], in_offset=None,
                        element_offset=h0,
                        compute_op=mybir.AluOpType.add)

    nc.compile()
    return nc


def pack_inputs(x, Wr, br, W1, b1, W2, b2, T, D, H, E, C, nd=256, n_cores=8):
    """Host-side shard + weight packing. Returns per-core in_maps."""
    DK, HJ, NH = D // P, H // P, D // nd
    x = np.ascontiguousarray(np.asarray(x, np.float32))
    Wr = np.asarray(Wr, np.float32)
    br = np.ascontiguousarray(np.asarray(br, np.float32))
    W1 = np.asarray(W1, np.float32)
    b1 = np.ascontiguousarray(np.asarray(b1, np.float32))
    W2 = np.asarray(W2, np.float32)
    b2 = np.ascontiguousarray(np.asarray(b2, np.float32))

    wrp = np.ascontiguousarray(Wr.reshape(DK, P, E).transpose(1, 0, 2))
    b1p = np.ascontiguousarray(b1.reshape(E, HJ, P).transpose(0, 2, 1))
    w1p = np.ascontiguousarray(
        W1.reshape(E, DK, P, HJ, P).transpose(0, 3, 2, 1, 4).astype(np.float16))
    w2p = np.ascontiguousarray(
        W2.reshape(E, HJ, P, NH, nd).transpose(0, 3, 2, 1, 4).astype(np.float16))

    Tsh = T // n_cores
    return [{
        "x": x[c * Tsh:(c + 1) * Tsh],
        "wrp": wrp, "br": br, "w1p": w1p, "b1p": b1p, "w2p": w2p, "b2": b2,
    } for c in range(n_cores)]


# ---------------- host wrapper ----------------

N_CORES = 8
T_FULL, D_FULL, H_FULL, E_FULL, C_FULL = 32768, 1024, 4096, 8, 1280

_nc_cache = {}


def _get_nc():
    if "full" not in _nc_cache:
        _nc_cache["full"] = build_moe(T_FULL // N_CORES, D_FULL, H_FULL,
                                      E_FULL, C_FULL)
    return _nc_cache["full"]


def kernel(x, Wr, br, W1, b1, W2, b2, _trace=False):
    from concourse.bass_utils import run_bass_kernel_spmd

    Tsh = T_FULL // N_CORES
    in_maps = pack_inputs(x, Wr, br, W1, b1, W2, b2,
                          T_FULL, D_FULL, H_FULL, E_FULL, C_FULL,
                          n_cores=N_CORES)
    nc = _get_nc()
    kw = dict(trace=True) if _trace else {}
    res = run_bass_kernel_spmd(nc, in_maps, core_ids=list(range(N_CORES)), **kw)
    outp = np.concatenate([res.results[c]["out"][1:1 + Tsh]
                           for c in range(N_CORES)], axis=0)
    if _trace:
        return outp, res
    return outp


# revision 12
# speedup vs baseline: 1.1601x; 1.1601x over previous
"""Mixture-of-Experts (top-2 of 8) Trainium2 Bass kernel.

Strategy: data-parallel over tokens — each of the 8 NeuronCores owns a
contiguous shard of 4096 tokens and computes its shard's full output.

Per core, on device:
  Phase R (routing): per 128-token tile, compute router logits on PE
    (PE-transpose + matmul), derive top-2 combine weights (softmax
    renormalized over the top-2 — the full-softmax denominator cancels),
    compute each token's slot in its experts' dispatch buffers with a
    matmul prefix-sum (strict-upper-triangular ones matrix + running
    base), then indirect-DMA-scatter the fp16 token row into per-expert
    compact buffers `xd` and (weight, token-id) rows into `meta`.
  Phase E (experts): per expert, DMA-transpose-load dispatched tokens as
    xT [d, slot], hT = gelu(W1.T @ xT + b1) with W1 streamed in j-blocks,
    out = hT.T @ W2 + b2 with W2 resident in halves, scale rows by the
    combine weight, indirect-DMA-scatter-accumulate into `out`.

Output row 0 is a trash row: pad slots carry id=0 / weight=0, so their
garbage lands there. Token t lives at out row 1+t. Biases are applied as
K=1 rank-1 matmuls. Weights are pre-packed and pre-cast to fp16 blocked
layouts on the host so every device weight DMA is contiguous per
partition.
"""

import os
import sys
from contextlib import ExitStack

import numpy as np

for _p in ("/opt/trn_rl_repo", "/root/.axon_site/_ro/trn_rl_repo"):
    if os.path.isdir(_p) and _p not in sys.path:
        sys.path.append(_p)

import concourse.bass as bass
import concourse.bacc as bacc
import concourse.tile as tile
from concourse import mybir
from concourse.masks import make_identity, make_upper_triangular

F32 = mybir.dt.float32
F16 = mybir.dt.float16
I32 = mybir.dt.int32
P = 128


def build_moe(T, D, H, E, C, act="Gelu", nd=256, mc=512):
    """Per-core MoE program. T tokens, D model, H hidden, E experts,
    C capacity/expert (mult of 128). nd = W2 d-chunk, mc = mm1 token chunk."""
    assert T % P == 0 and D % P == 0 and H % P == 0 and C % P == 0
    DK, HJ, SC, NT, NH = D // P, H // P, C // P, T // P, D // nd
    act_fn = getattr(mybir.ActivationFunctionType, act)

    nc = bacc.Bacc("TRN2", target_bir_lowering=False, debug=False)

    x = nc.declare_dram_parameter("x", [T, D], F32, isOutput=False)
    wrp = nc.declare_dram_parameter("wrp", [P, DK, E], F32, isOutput=False)
    br = nc.declare_dram_parameter("br", [E], F32, isOutput=False)
    w1p = nc.declare_dram_parameter("w1p", [E, HJ, P, DK, P], F16, isOutput=False)
    b1p = nc.declare_dram_parameter("b1p", [E, P, HJ], F32, isOutput=False)
    w2p = nc.declare_dram_parameter("w2p", [E, NH, P, HJ, nd], F16, isOutput=False)
    b2 = nc.declare_dram_parameter("b2", [E, D], F32, isOutput=False)
    out = nc.declare_dram_parameter("out", [1 + T + P, D], F32, isOutput=True)

    xd = nc.dram_tensor("xd", [E * C, D], F16)        # dispatched tokens
    meta = nc.dram_tensor("meta", [E * C, 2], F32)    # (combine w, 1 + token id)

    with ExitStack() as ctx:
        tc = ctx.enter_context(tile.TileContext(nc))
        cst = ctx.enter_context(tc.tile_pool(name="cst", bufs=1))
        pR2 = ctx.enter_context(tc.tile_pool(name="pR2", bufs=2))
        psA = ctx.enter_context(tc.tile_pool(name="psA", bufs=3, space="PSUM"))
        psB = ctx.enter_context(tc.tile_pool(name="psB", bufs=3, space="PSUM"))
        xt1 = ctx.enter_context(tc.tile_pool(name="xt1", bufs=1))
        ht1 = ctx.enter_context(tc.tile_pool(name="ht1", bufs=1))
        w1pl = ctx.enter_context(tc.tile_pool(name="w1pl", bufs=3))
        w2pl = ctx.enter_context(tc.tile_pool(name="w2pl", bufs=2))
        otp = ctx.enter_context(tc.tile_pool(name="otp", bufs=3))
        sm2 = ctx.enter_context(tc.tile_pool(name="sm2", bufs=2))

        # ---- constants ----
        su = cst.tile([P, P], F32)  # (made before ident: PE's first pool-wait covers both)  # su[m, n] = 1 iff m < n
        make_upper_triangular(nc, su[:], val=1.0, diag=False)
        ident = cst.tile([P, P], F32)
        make_identity(nc, ident[:])
        ones_r = cst.tile([1, max(P, nd, mc, E)], F32)
        nc.vector.memset(ones_r[:], 1.0)
        ones_c = cst.tile([P, 1], F32)
        nc.vector.memset(ones_c[:], 1.0)
        ecol = cst.tile([P, E], F32)  # ecol[p, e] = e * C
        nc.gpsimd.iota(ecol[:], pattern=[[1, E]], base=0, channel_multiplier=0,
                       allow_small_or_imprecise_dtypes=True)
        nc.vector.tensor_scalar_mul(out=ecol[:], in0=ecol[:], scalar1=float(C))
        wr_sb = cst.tile([P, DK, E], F32)
        nc.sync.dma_start(out=wr_sb[:], in_=wrp[:])
        br_sb = cst.tile([1, E], F32)
        nc.sync.dma_start(out=br_sb[:1, :], in_=br[None, :])
        zt = cst.tile([P, D], F32)
        nc.vector.memset(zt[:], 0.0)

        # ---- zero-init out and meta ----
        n_zero = (1 + T + P + P - 1) // P
        for i in range(n_zero):
            r0 = i * P
            r1_ = min(r0 + P, 1 + T + P)
            nc.scalar.dma_start(out=out[r0:r1_, :], in_=zt[: r1_ - r0, :])
        npp = E * C // P  # meta rows per partition
        nc.scalar.dma_start(out=meta[:].rearrange("(p n) c -> p (n c)", p=P),
                          in_=zt[:, : npp * 2])
        # token ids for all tiles: tok_all[p, it] = 1 + it*128 + p
        tok_all = cst.tile([P, 512], I32)
        nc.gpsimd.iota(tok_all[:], pattern=[[P, 512]], base=1, channel_multiplier=1)
        b1_all = cst.tile([P, E, HJ], F32)
        nc.sync.dma_start(out=b1_all[:], in_=b1p[:].rearrange("e p j -> p e j"))
        b2_all = cst.tile([1, E, D], F32)
        nc.sync.dma_start(out=b2_all[:1], in_=b2[None, :, :])
        zt16 = cst.tile([P, D], F16)
        nc.vector.memset(zt16[:], 0.0)
        for i in range(E * C // P):
            nc.scalar.dma_start(out=xd[i * P:(i + 1) * P, :], in_=zt16[:])

        # ---- Phase R: routing + dispatch ----
        bc_reg = nc.gpsimd.to_reg(E * C - 1)
        base_prev = None
        for it in range(NT):
            x_t = pR2.tile([P, D], F32, tag="x_t")
            nc.sync.dma_start(out=x_t[:], in_=x[it * P:(it + 1) * P, :])
            x16 = pR2.tile([P, D], F16, tag="x16")
            nc.vector.tensor_copy(out=x16[:], in_=x_t[:])

            xTr = pR2.tile([P, DK, P], F32, tag="xTr")
            for dk in range(DK):
                tr_ps = psA.tile([P, P], F32, tag="mmout", space="PSUM")
                nc.tensor.transpose(out=tr_ps[:], in_=x_t[:, dk * P:(dk + 1) * P],
                                    identity=ident[:])
                nc.vector.tensor_copy(out=xTr[:, dk, :], in_=tr_ps[:])

            lg_ps = psB.tile([P, E], F32, tag="small", space="PSUM")
            for dk in range(DK):
                nc.tensor.matmul(lg_ps[:], lhsT=xTr[:, dk, :], rhs=wr_sb[:, dk, :],
                                 start=(dk == 0), stop=False)
            nc.tensor.matmul(lg_ps[:], lhsT=ones_r[:1, :P], rhs=br_sb[:1, :],
                             start=False, stop=True)
            lg = pR2.tile([P, E], F32, tag="lg")
            nc.vector.tensor_copy(out=lg[:], in_=lg_ps[:])

            # top-2 combine weights
            m1 = pR2.tile([P, 1], F32, tag="m1")
            nc.vector.reduce_max(out=m1[:], in_=lg[:], axis=mybir.AxisListType.X)
            is1 = pR2.tile([P, E], F32, tag="is1")
            nc.vector.tensor_scalar(out=is1[:], in0=lg[:], scalar1=m1[:, :1],
                                    scalar2=None, op0=mybir.AluOpType.is_equal)
            lm = pR2.tile([P, E], F32, tag="lm")
            nc.vector.tensor_scalar(out=lm[:], in0=is1[:], scalar1=-1e30, scalar2=None,
                                    op0=mybir.AluOpType.mult)
            nc.vector.tensor_add(out=lm[:], in0=lm[:], in1=lg[:])
            m2 = pR2.tile([P, 1], F32, tag="m2")
            nc.vector.reduce_max(out=m2[:], in_=lm[:], axis=mybir.AxisListType.X)
            m1n = pR2.tile([P, 1], F32, tag="m1n")
            nc.vector.tensor_scalar_mul(out=m1n[:], in0=m1[:], scalar1=-1.0)
            ex = pR2.tile([P, E], F32, tag="ex")
            nc.scalar.activation(out=ex[:], in_=lg[:],
                                 func=mybir.ActivationFunctionType.Exp,
                                 bias=m1n[:, :1], scale=1.0)
            sel = pR2.tile([P, E], F32, tag="sel")
            nc.vector.tensor_scalar(out=sel[:], in0=lg[:], scalar1=m2[:, :1],
                                    scalar2=None, op0=mybir.AluOpType.is_ge)
            w = pR2.tile([P, E], F32, tag="w")
            nc.vector.tensor_mul(out=w[:], in0=ex[:], in1=sel[:])
            den = pR2.tile([P, 1], F32, tag="den")
            nc.vector.reduce_sum(out=den[:], in_=w[:], axis=mybir.AxisListType.X)
            rcp = pR2.tile([P, 1], F32, tag="rcp")
            nc.vector.reciprocal(out=rcp[:], in_=den[:])
            comb = pR2.tile([P, E], F32, tag="comb")
            nc.vector.tensor_scalar_mul(out=comb[:], in0=w[:], scalar1=rcp[:, :1])

            # pos[n, e] = base[e] + sum_{m<n in tile} sel[m, e]
            pos_ps = psB.tile([P, E], F32, tag="small", space="PSUM")
            nc.tensor.matmul(pos_ps[:], lhsT=su[:], rhs=sel[:], start=True,
                             stop=(base_prev is None))
            if base_prev is not None:
                nc.tensor.matmul(pos_ps[:], lhsT=ones_r[:1, :P], rhs=base_prev[:1, :],
                                 start=False, stop=True)
            pos = pR2.tile([P, E], F32, tag="pos")
            nc.vector.tensor_copy(out=pos[:], in_=pos_ps[:])

            # base_new = base_prev + colsum(sel)
            cs_ps = psB.tile([1, E], F32, tag="small", space="PSUM")
            nc.tensor.matmul(cs_ps[:1, :], lhsT=ones_c[:, :1], rhs=sel[:],
                             start=True, stop=True)
            base_new = pR2.tile([1, E], F32, tag="base")
            if base_prev is not None:
                nc.vector.tensor_add(out=base_new[:1, :], in0=base_prev[:1, :],
                                     in1=cs_ps[:1, :])
            else:
                nc.vector.tensor_copy(out=base_new[:1, :], in_=cs_ps[:1, :])
            base_prev = base_new

            # offsets (with capacity-overflow guard -> OOB skip)
            offm = pR2.tile([P, E], F32, tag="offm")
            ovf = pR2.tile([P, E], F32, tag="ovf")
            nc.vector.tensor_scalar(out=ovf[:], in0=pos[:], scalar1=float(C) - 0.5,
                                    scalar2=1e9, op0=mybir.AluOpType.is_ge,
                                    op1=mybir.AluOpType.mult)
            nc.vector.tensor_add(out=offm[:], in0=pos[:], in1=ecol[:])
            nc.vector.tensor_add(out=offm[:], in0=offm[:], in1=ovf[:])

            r1 = pR2.tile([P, E], F32, tag="r1")
            nc.vector.tensor_sub(out=r1[:], in0=sel[:], in1=is1[:])

            tokf = pR2.tile([P, 1], F32, tag="tokf")
            nc.vector.tensor_copy(out=tokf[:], in_=tok_all[:, it:it + 1])

            for rank, mask in ((0, is1), (1, r1)):
                tmp = pR2.tile([P, E], F32, tag=f"tmp{rank}")
                offr = pR2.tile([P, 1], F32, tag=f"offr{rank}")
                nc.vector.tensor_mul(out=tmp[:], in0=offm[:], in1=mask[:])
                nc.vector.reduce_sum(out=offr[:], in_=tmp[:], axis=mybir.AxisListType.X)
                offi = pR2.tile([P, 1], I32, tag=f"offi{rank}")
                nc.vector.tensor_copy(out=offi[:], in_=offr[:])
                wr_t = pR2.tile([P, 1], F32, tag=f"wr_t{rank}")
                nc.vector.tensor_mul(out=tmp[:], in0=comb[:], in1=mask[:])
                nc.vector.reduce_sum(out=wr_t[:], in_=tmp[:], axis=mybir.AxisListType.X)
                mt = pR2.tile([P, 2], F32, tag=f"mt{rank}")
                nc.vector.tensor_copy(out=mt[:, 0:1], in_=wr_t[:])
                nc.vector.tensor_copy(out=mt[:, 1:2], in_=tokf[:])
                nc.gpsimd.indirect_dma_start(
                    out=meta[:],
                    out_offset=bass.IndirectOffsetOnAxis(ap=offi[:, :1], axis=0),
                    in_=mt[:], in_offset=None,
                    bounds_check=bc_reg, oob_is_err=False)
                nc.gpsimd.indirect_dma_start(
                    out=xd[:],
                    out_offset=bass.IndirectOffsetOnAxis(ap=offi[:, :1], axis=0),
                    in_=x16[:], in_offset=None,
                    bounds_check=bc_reg, oob_is_err=False)

        # ---- Phase E: per-expert MLP ----
        mchunks = []
        s0 = 0
        while s0 < C:
            mchunks.append((s0, min(mc, C - s0)))
            s0 += mc

        for e in range(E):
            meta_sb = sm2.tile([P, SC, 2], F32, tag="meta_sb")
            nc.sync.dma_start(
                out=meta_sb[:],
                in_=meta[e * C:(e + 1) * C, :].rearrange("(s p) c -> p s c", p=P))
            idx_i = sm2.tile([P, SC], I32, tag="idx_i")
            nc.vector.tensor_copy(out=idx_i[:], in_=meta_sb[:, :, 1])

            xT = xt1.tile([P, DK, C], F16, tag="xT")
            for dk in range(DK):
                nc.sync.dma_start(
                    out=xT[:, dk, :],
                    in_=xd[e * C:(e + 1) * C, dk * P:(dk + 1) * P],
                    transpose=True)

            hT = ht1.tile([P, HJ, C], F16, tag="hT")
            for j in range(HJ):
                w1j = w1pl.tile([P, DK, P], F16, tag="w1j")
                nc.sync.dma_start(out=w1j[:], in_=w1p[e, j])
                for (c0, ns) in mchunks:
                    ph = psA.tile([P, mc], F32, tag="mmout", space="PSUM")
                    for dk in range(DK):
                        nc.tensor.matmul(ph[:, :ns], lhsT=w1j[:, dk, :],
                                         rhs=xT[:, dk, c0:c0 + ns],
                                         start=(dk == 0), stop=(dk == DK - 1))
                    nc.scalar.activation(out=hT[:, j, c0:c0 + ns], in_=ph[:, :ns],
                                         func=act_fn, bias=b1_all[:, e, j:j + 1], scale=1.0)

            for hi in range(NH):
                h0 = hi * nd
                w2h = w2pl.tile([P, HJ, nd], F16, tag="w2h")
                nc.sync.dma_start(out=w2h[:], in_=w2p[e, hi])
                for s in range(SC):
                    po = psA.tile([P, nd], F32, tag="mmout", space="PSUM")
                    for j in range(HJ):
                        nc.tensor.matmul(po[:], lhsT=hT[:, j, s * P:(s + 1) * P],
                                         rhs=w2h[:, j, :], start=(j == 0), stop=False)
                    nc.tensor.matmul(po[:], lhsT=ones_r[:1, :P],
                                     rhs=b2_all[:1, e, h0:h0 + nd], start=False, stop=True)
                    ot = otp.tile([P, nd], F32, tag="ot")
                    nc.vector.tensor_scalar_mul(out=ot[:], in0=po[:],
                                                scalar1=meta_sb[:, s, 0:1])
                    nc.gpsimd.indirect_dma_start(
                        out=out[:],
                        out_offset=bass.IndirectOffsetOnAxis(ap=idx_i[:, s:s + 1], axis=0),
                        in_=ot[:], in_offset=None,
                        element_offset=h0,
                        compute_op=mybir.AluOpType.add)

    nc.compile()
    return nc


def pack_inputs(x, Wr, br, W1, b1, W2, b2, T, D, H, E, C, nd=256, n_cores=8):
    """Host-side shard + weight packing. Returns per-core in_maps."""
    DK, HJ, NH = D // P, H // P, D // nd
    x = np.ascontiguousarray(np.asarray(x, np.float32))
    Wr = np.asarray(Wr, np.float32)
    br = np.ascontiguousarray(np.asarray(br, np.float32))
    W1 = np.asarray(W1, np.float32)
    b1 = np.ascontiguousarray(np.asarray(b1, np.float32))
    W2 = np.asarray(W2, np.float32)
    b2 = np.ascontiguousarray(np.asarray(b2, np.float32))

    wrp = np.ascontiguousarray(Wr.reshape(DK, P, E).transpose(1, 0, 2))
    b1p = np.ascontiguousarray(b1.reshape(E, HJ, P).transpose(0, 2, 1))
    w1p = np.ascontiguousarray(
        W1.reshape(E, DK, P, HJ, P).transpose(0, 3, 2, 1, 4).astype(np.float16))
    w2p = np.ascontiguousarray(
        W2.reshape(E, HJ, P, NH, nd).transpose(0, 3, 2, 1, 4).astype(np.float16))

    Tsh = T // n_cores
    return [{
        "x": x[c * Tsh:(c + 1) * Tsh],
        "wrp": wrp, "br": br, "w1p": w1p, "b1p": b1p, "w2p": w2p, "b2": b2,
    } for c in range(n_cores)]


# ---------------- host wrapper ----------------

N_CORES = 8
T_FULL, D_FULL, H_FULL, E_FULL, C_FULL = 32768, 1024, 4096, 8, 1280

_nc_cache = {}


def _get_nc():
    if "full" not in _nc_cache:
        _nc_cache["full"] = build_moe(T_FULL // N_CORES, D_FULL, H_FULL,
                                      E_FULL, C_FULL)
    return _nc_cache["full"]


def kernel(x, Wr, br, W1, b1, W2, b2, _trace=False):
    from concourse.bass_utils import run_bass_kernel_spmd

    Tsh = T_FULL // N_CORES
    in_maps = pack_inputs(x, Wr, br, W1, b1, W2, b2,
                          T_FULL, D_FULL, H_FULL, E_FULL, C_FULL,
                          n_cores=N_CORES)
    nc = _get_nc()
    kw = dict(trace=True) if _trace else {}
    res = run_bass_kernel_spmd(nc, in_maps, core_ids=list(range(N_CORES)), **kw)
    outp = np.concatenate([res.results[c]["out"][1:1 + Tsh]
                           for c in range(N_CORES)], axis=0)
    if _trace:
        return outp, res
    return outp
